# revision 13
# baseline (speedup 1.0000x reference)
"""Bass/Tile TRN2 kernel for nn_Disen_GAT_For_Multi_Aspect (v3).

Contract: kernel(**inputs) takes FULL fp32 numpy inputs (keys as in
reference.setup_inputs()) and returns the FULL [B, A, H] fp32 output.

Strategy
--------
Data-parallel over batch B across the 8 cores (1 batch row / core, A=4
aspects per core).  The reference collapses algebraically:

  q = Wq^T asp + bq;  u = tA q; v = tB q; y = W1b v; a3 = W1a^T q
  w[a,k] = sum_{i,j} q[a,i] v[a,j] T1[i,j,k]
  G = Wk @ [q|w|y|u]                    (per aspect, [D,4])
  logit rows vs raw streams:  st = (Wk q).T_n, sx* = (Wk{q,w,y}).X_n,
                              sd = (Wk u).Dp_n
  V_W = Wv^T X + bv, V_T = Wv^T T + bv  ([H,N] per aspect)
  att_z[h] = sum_n att_n V_W[h,n] V_T[h,n]

PE-centric v3 layout:
 * T1 pass computes w DIRECTLY: 128 accumulating matmuls with rank-1
   lhsT chunks qv_j = q (.) v_j (fp8), rhs = T1[:, j-block] (fp8).
 * Per aspect the 5 logit rows land in ONE PSUM bank via tile_position
   col placement (X-rows@0, T-rows@32, Dp-rows@64, neg-row@96), padded
   to M=32 so every PSUM partition is written.  Row matmuls run in
   fp8 DoubleRow mode (K=256 chunk pairs): device-side single-copy
   casts of the bf16 X|T stream and of G to fp8.  One full-width bf16
   copy of the row bank, then softmax as: combo-matmul [128x3] -> ACT
   Exp(bias,scale,accum z) -> reciprocal -> alpha-broadcast matmul =
   att replicated on 128 partitions -> multiply+reduce vs V_W*V_T.
 * Streams: X,T bf16 (V path); rows read fp8 casts; Dp fp8; T1 fp8.
 * q-chain in bf16; constants split hot (q-chain) / cold (stream
   weights) so the first matmul issues as early as possible.
"""

import contextlib
import ctypes
import sys
import types

import numpy as np
import ml_dtypes

import concourse.bacc as bacc
import concourse.mybir as mybir
import concourse.tile as tile
from concourse.bass_utils import run_bass_kernel_spmd

B, A, N, D, H = 8, 4, 512, 1024, 128
SCALE = float(np.sqrt(H))
NCORES = 8
DC = D // H  # 8 contraction chunks of 128

F32 = mybir.dt.float32
BF16 = mybir.dt.bfloat16
F8 = mybir.dt.float8e4
BF = ml_dtypes.bfloat16
E4 = ml_dtypes.float8_e4m3fn
AF = mybir.ActivationFunctionType
OP = mybir.AluOpType
DR = mybir.MatmulPerfMode.DoubleRow

# cpackh (bf16, hot: q-chain weights) column layout
CH_WQ = 0              # [128, 8, 128] Wq chunk-packed
CH_ASP = 1024          # [128, 8, 4] aspect^T chunk-packed
CH_TAT = 1056          # trans_W[:H].T
CH_TBT = 1184          # trans_W[H:].T
CH_W1A = 1312          # W1_W[:H] (raw)
CH_W1BT = 1440         # W1_W[H:].T
CH_W = 1568
# cpackf (f32, small) column layout
CF_BQROW = 0           # rows 0-3: bq as a row [4, 128]
CF_BIAS = 128          # cols: bq|bk|bv|W1_b|trans_b
CF_COMBW = 133         # rows 0-2: comb_w column
CF_MASK4 = 134         # rows 0-3: eye(4) columns
CF_M01 = 138           # [1/S, 1/S, 0] column (rows 0-2)
CF_M2 = 139            # [0, 0, 1/S] column
CF_W = 140
# cpackb (bf16, cold: stream weights) column layout
CB_WKT = 0             # [128, 1024]  Wk^T
CB_WV = 1024           # [128, 8, 128] Wv chunk-packed
CB_MROW = 2048         # rows 0-3: fmask replicated [4, 512]
CB_COMBO = 2560        # [128, 3] combo matrix
CB_E0 = 2563           # [4, 32] one-hot row-0 picker (neg MM lhsT)
CB_ID4 = 2595          # [4, 4] identity
CB_W = 2600

LAST_RESULTS = None  # test harness peeks at this


def _build(ncores=NCORES):
    nc = bacc.Bacc("TRN2", target_bir_lowering=False, debug=False,
                   num_devices=ncores)

    xt = nc.dram_tensor("xt", [A, 128, DC, 2, N], BF16, kind="ExternalInput")
    dp8 = nc.dram_tensor("dp8", [A, 128, DC // 2, 2, N], F8,
                         kind="ExternalInput")
    t1f = nc.dram_tensor("t1f", [H, H * H], F8, kind="ExternalInput")
    cpackh = nc.dram_tensor("cpackh", [128, CH_W], BF16, kind="ExternalInput")
    cpackf = nc.dram_tensor("cpackf", [128, CF_W], F32, kind="ExternalInput")
    cpackb = nc.dram_tensor("cpackb", [128, CB_W], BF16, kind="ExternalInput")
    out = nc.dram_tensor("out", [H, A], F32, kind="ExternalOutput")

    inv_s = 1.0 / SCALE

    with tile.TileContext(nc) as tc:
        with (
            tc.tile_pool(name="const", bufs=1) as cp,
            tc.tile_pool(name="t1s", bufs=2) as tp,
            tc.tile_pool(name="xts", bufs=2) as xp,
            tc.tile_pool(name="work", bufs=2) as wp,
            tc.tile_pool(name="vzone", bufs=4, space="PSUM") as vps,
            tc.tile_pool(name="rzone", bufs=2, space="PSUM") as rps,
            tc.tile_pool(name="szone", bufs=2, space="PSUM") as sps,
        ):
            # ---- input DMAs, spread across the three DMA sequencers ---
            cph = cp.tile([128, CH_W], BF16, tag="cph")
            nc.sync.dma_start(out=cph, in_=cpackh.ap())
            cpf = cp.tile([128, CF_W], F32, tag="cpf")
            nc.scalar.dma_start(out=cpf, in_=cpackf.ap())
            t1sb = []
            for i in range(4):
                t = tp.tile([128, 4096], F8, tag="t1", bufs=4)
                eng = nc.scalar if i % 2 == 0 else nc.sync
                eng.dma_start(out=t, in_=t1f.ap()[:, 4096 * i:4096 * (i + 1)])
                t1sb.append(t)
            cpb = cp.tile([128, CB_W], BF16, tag="cpb")
            nc.scalar.dma_start(out=cpb, in_=cpackb.ap())
            xa_t = {}
            dp_t = {}
            xeng = [nc.gpsimd, nc.sync, nc.scalar, nc.gpsimd]
            deng = [nc.gpsimd, nc.sync, nc.sync, nc.scalar]
            HC = DC // 2
            for a in range(A):
                halves = []
                for hf in range(2):
                    xh = xp.tile([128, HC, 2, N], BF16, tag="xt", bufs=4)
                    xeng[a].dma_start(
                        out=xh, in_=xt.ap()[a, :, hf * HC:(hf + 1) * HC])
                    halves.append(xh)
                da = xp.tile([128, DC // 2, 2, N], F8, tag="dp")
                deng[a].dma_start(out=da, in_=dp8.ap()[a])
                xa_t[a] = halves
                dp_t[a] = da

            # ---- constant views ---------------------------------------
            wq_v = cph[:, CH_WQ:CH_WQ + DC * H].rearrange(
                "p (c h) -> p c h", c=DC)
            asp_v = cph[:, CH_ASP:CH_ASP + DC * A].rearrange(
                "p (c a) -> p c a", c=DC)
            tat_sb = cph[:, CH_TAT:CH_TAT + H]
            tbt_sb = cph[:, CH_TBT:CH_TBT + H]
            w1a_sb = cph[:, CH_W1A:CH_W1A + H]
            w1bt_sb = cph[:, CH_W1BT:CH_W1BT + H]
            bqrow = cpf[0:4, CF_BQROW:CF_BQROW + H]
            bq_c = cpf[:, CF_BIAS + 0:CF_BIAS + 1]
            bk_c = cpf[:, CF_BIAS + 1:CF_BIAS + 2]
            bv_c = cpf[:, CF_BIAS + 2:CF_BIAS + 3]
            b1_c = cpf[:, CF_BIAS + 3:CF_BIAS + 4]
            tb_c = cpf[:, CF_BIAS + 4:CF_BIAS + 5]
            combw3 = cpf[0:3, CF_COMBW:CF_COMBW + 1]
            mask4 = cpf[0:4, CF_MASK4:CF_MASK4 + 4]
            m01_c = cpf[0:3, CF_M01:CF_M01 + 1]
            m2_c = cpf[0:3, CF_M2:CF_M2 + 1]
            wkt_sb = cpb[:, CB_WKT:CB_WKT + D]
            wv_v = cpb[:, CB_WV:CB_WV + DC * H].rearrange(
                "p (c h) -> p c h", c=DC)
            mrow4 = cpb[0:4, CB_MROW:CB_MROW + N]
            combo_m = cpb[:, CB_COMBO:CB_COMBO + 3]
            id4 = cpb[0:4, CB_ID4:CB_ID4 + 4]

            ones_col = cp.tile([128, 1], F32, tag="ones_col")
            nc.vector.memset(ones_col, 1.0)
            ones3r = cp.tile([3, 128], BF16, tag="ones3r")
            nc.vector.memset(ones3r, 1.0)

            # neg rows: [32, N] with row 0 = -1e30*(1-m), rest zero
            negfull = cp.tile([32, N], BF16, tag="negfull")
            nc.vector.memset(negfull, 0.0)
            nc.vector.tensor_scalar(negfull[0:1, :], mrow4[0:1, :], 1e30,
                                    1e30, op0=OP.mult, op1=OP.subtract)

            # ---- q chain (bf16 matmuls, fp32 psum) --------------------
            ps_q = sps.tile([H, A], F32, tag="s")
            for c in range(DC):
                nc.tensor.matmul(ps_q, lhsT=wq_v[:, c, :], rhs=asp_v[:, c, :],
                                 start=(c == 0), stop=(c == DC - 1))
            q4 = cp.tile([H, A], F32, tag="q4")
            nc.scalar.activation(q4, ps_q, AF.Identity, bias=bq_c)
            q4b = cp.tile([H, A], BF16, tag="q4b")
            nc.vector.tensor_copy(q4b, q4)

            ps_qT = sps.tile([A, H], F32, tag="s")
            for c in range(DC):
                nc.tensor.matmul(ps_qT, lhsT=asp_v[:, c, :], rhs=wq_v[:, c, :],
                                 start=(c == 0), stop=(c == DC - 1))
            qTb = cp.tile([A, H], BF16, tag="qTb")
            nc.vector.tensor_tensor(qTb, ps_qT, bqrow, op=OP.add)

            ps_s = sps.tile([H, A], F32, tag="s")
            nc.tensor.matmul(ps_s, lhsT=tbt_sb, rhs=q4b, start=True, stop=True)
            v4 = cp.tile([H, A], F32, tag="v4")
            nc.vector.tensor_copy(v4, ps_s)
            v4b = cp.tile([H, A], BF16, tag="v4b")
            nc.vector.tensor_copy(v4b, ps_s)

            ps_vT = sps.tile([A, H], F32, tag="s")
            nc.tensor.matmul(ps_vT, lhsT=q4b, rhs=tbt_sb, start=True,
                             stop=True)
            vTb = cp.tile([A, H], BF16, tag="vTb")
            nc.vector.tensor_copy(vTb, ps_vT)

            # qwyu: aspect-major columns [q|w|y|u] per aspect, bf16
            qwyu = cp.tile([H, 16], BF16, tag="qwyu")
            qwv = qwyu.rearrange("p (a v) -> p a v", a=4)
            nc.vector.tensor_copy(qwv[:, :, 0], q4)

            ps_s = sps.tile([H, A], F32, tag="s")
            nc.tensor.matmul(ps_s, lhsT=tat_sb, rhs=q4b, start=True, stop=True)
            u4 = cp.tile([H, A], F32, tag="u4")
            nc.vector.tensor_copy(u4, ps_s)
            nc.vector.tensor_copy(qwv[:, :, 3], ps_s)

            ps_s = sps.tile([H, A], F32, tag="s")
            nc.tensor.matmul(ps_s, lhsT=w1bt_sb, rhs=v4b, start=True,
                             stop=True)
            y4 = cp.tile([H, A], F32, tag="y4")
            nc.vector.tensor_copy(y4, ps_s)
            nc.vector.tensor_copy(qwv[:, :, 2], ps_s)

            ps_s = sps.tile([H, A], F32, tag="s")
            nc.tensor.matmul(ps_s, lhsT=w1a_sb, rhs=q4b, start=True, stop=True)
            a3q = cp.tile([H, A], F32, tag="a3q")
            nc.vector.tensor_copy(a3q, ps_s)

            # ---- qv outer products (masked K=4), cast fp8 -------------
            ps_qv = sps.tile([128, 4 * H], F32, tag="s")
            for a in range(A):
                vTm = wp.tile([A, H], BF16, tag="vTm")
                nc.vector.tensor_scalar_mul(vTm, vTb, mask4[:, a:a + 1])
                nc.tensor.matmul(ps_qv[:, a * H:(a + 1) * H], lhsT=qTb,
                                 rhs=vTm, start=True, stop=True)
            qv8 = cp.tile([128, 4 * H], F8, tag="qv8")
            nc.vector.tensor_copy(qv8, ps_qv)
            qv8v = qv8.rearrange("p (a j) -> p j a", a=4)

            # ---- helpers for interleaved emission ---------------------
            vps_t = {}
            vv_t = {}
            pp_t = {}
            rm_t = {}

            def emit_t1_block(blk, ps_w):
                for j in range(32 * blk, 32 * blk + 32):
                    nc.tensor.matmul(ps_w, lhsT=qv8v[:, j, :],
                                     rhs=t1sb[j // 32][:, (j % 32) * H:
                                                       (j % 32 + 1) * H],
                                     start=(j == 0), stop=(j == H - 1))

            def emit_v_half(a, hf):
                if hf == 0:
                    ps_vw = vps.tile([H, N], F32, tag="v")
                    ps_vt = vps.tile([H, N], F32, tag="v")
                    vps_t[a] = (ps_vw, ps_vt)
                ps_vw, ps_vt = vps_t[a]
                xh = xa_t[a][hf]
                for c in range(HC):
                    cc = hf * HC + c
                    nc.tensor.matmul(ps_vw, lhsT=wv_v[:, cc, :],
                                     rhs=xh[:, c, 0, :], start=(cc == 0),
                                     stop=(cc == DC - 1))
                    nc.tensor.matmul(ps_vt, lhsT=wv_v[:, cc, :],
                                     rhs=xh[:, c, 1, :], start=(cc == 0),
                                     stop=(cc == DC - 1))
                if hf == 1:
                    vv = wp.tile([H, 2 * N], F32, tag="vv")
                    nc.scalar.activation(vv[:, 0:N], ps_vw, AF.Identity,
                                         bias=bv_c)
                    nc.scalar.activation(vv[:, N:2 * N], ps_vt, AF.Identity,
                                         bias=bv_c)
                    pprod = wp.tile([H, N], F32, tag="pprod")
                    nc.vector.tensor_mul(pprod, vv[:, 0:N], vv[:, N:2 * N])
                    vv_t[a] = vv
                    pp_t[a] = pprod

            def emit_rows(a):
                da = dp_t[a]
                ps_rm = rps.tile([128, N], F32, tag="rows", bufs=1)
                for st in range(2):
                    for c in range(DC):
                        xh = xa_t[a][c // HC]
                        nc.tensor.matmul(ps_rm[32 * st:32 * st + 32, :],
                                         lhsT=gall[:, c, 4 * a:4 * a + 32],
                                         rhs=xh[:, c % HC, st, :],
                                         start=(c == 0), stop=(c == DC - 1),
                                         tile_position=(0, 32 * st))
                ps_rd = rps.tile([32, N], F32, tag="rowsd", bufs=1)
                for c2 in range(DC // 2):
                    nc.tensor.matmul(ps_rd,
                                     lhsT=g8p[:, c2, :, 4 * a:4 * a + 32],
                                     rhs=da[:, c2, :, :], start=(c2 == 0),
                                     stop=(c2 == DC // 2 - 1),
                                     perf_mode=DR)
                rm_t[a] = (ps_rm, ps_rd)

            def emit_epi(a):
                ps_rm, ps_rd = rm_t[a]
                # assemble rows bank: X@0, T@32, neg@64, Dp@96
                rows_bf = wp.tile([128, N], BF16, tag="rows_bf")
                nc.vector.tensor_copy(rows_bf[0:64, :], ps_rm[0:64, :])
                nc.scalar.copy(rows_bf[64:96, :], negfull)
                nc.vector.tensor_copy(rows_bf[96:128, :], ps_rd)
                ps_combo = sps.tile([3, N], F32, tag="s")
                nc.tensor.matmul(ps_combo, lhsT=combo_m, rhs=rows_bf,
                                 start=True, stop=True)
                e3 = wp.tile([3, N], BF16, tag="e3")
                z3 = wp.tile([3, 1], F32, tag="z3")
                nc.scalar.activation(e3, ps_combo, AF.Exp,
                                     bias=bias_all[:, a:a + 1], scale=inv_s,
                                     accum_out=z3)
                rz = wp.tile([3, 1], F32, tag="rz")
                nc.vector.reciprocal(rz, z3)
                alpha = wp.tile([3, 1], F32, tag="alpha")
                nc.vector.tensor_mul(alpha, rz, combw3)
                arep = wp.tile([3, H], BF16, tag="arep")
                nc.vector.tensor_scalar_mul(arep, ones3r, alpha)
                ps_att = sps.tile([H, N], F32, tag="s")
                nc.tensor.matmul(ps_att, lhsT=arep, rhs=e3,
                                 start=True, stop=True)
                scr = wp.tile([H, N], F32, tag="scr")
                nc.vector.tensor_mul(scr, ps_att, pp_t[a], )
                nc.vector.tensor_reduce(attz[:, a:a + 1], scr,
                                        axis=mybir.AxisListType.X,
                                        op=OP.add)

            attz = cp.tile([H, A], F32, tag="attz")

            # ---- T1 pass interleaved with aspect-0 V work -------------
            ps_w = sps.tile([A, H], F32, tag="s")
            emit_t1_block(0, ps_w)
            emit_t1_block(1, ps_w)
            emit_t1_block(2, ps_w)
            emit_v_half(0, 0)
            emit_t1_block(3, ps_w)

            wbf = cp.tile([A, H], BF16, tag="wbf")
            nc.vector.tensor_copy(wbf, ps_w)
            ps_tr = sps.tile([H, A], BF16, tag="s")
            nc.tensor.transpose(ps_tr, wbf, id4)
            nc.vector.tensor_copy(qwv[:, :, 1], ps_tr)

            # ---- scalar terms -> bias_all [3, A] ----------------------
            # groups: cbk | u.bk | w.bk | y.bk | a3.v | v.W1b | q.tb
            tmp28 = cp.tile([H, 28], F32, tag="tmp28")
            nc.vector.tensor_scalar_mul(tmp28[:, 0:4], q4, bk_c)
            nc.vector.tensor_scalar_mul(tmp28[:, 4:8], u4, bk_c)
            wcol = cp.tile([H, A], F32, tag="wcol")
            nc.vector.tensor_copy(wcol, ps_tr)
            nc.vector.tensor_scalar_mul(tmp28[:, 8:12], wcol, bk_c)
            nc.vector.tensor_scalar_mul(tmp28[:, 12:16], y4, bk_c)
            nc.vector.tensor_mul(tmp28[:, 16:20], a3q, v4)
            nc.vector.tensor_scalar_mul(tmp28[:, 20:24], v4, b1_c)
            nc.vector.tensor_scalar_mul(tmp28[:, 24:28], q4, tb_c)
            ps_c28 = sps.tile([1, 28], F32, tag="s")
            nc.tensor.matmul(ps_c28, lhsT=ones_col, rhs=tmp28,
                             start=True, stop=True)
            c28 = cp.tile([1, 28], F32, tag="c28")
            nc.vector.tensor_copy(c28, ps_c28)
            one13 = cp.tile([1, 3], F32, tag="one13")
            nc.vector.memset(one13, 1.0)
            ps_r3 = sps.tile([3, 28], F32, tag="s")
            nc.tensor.matmul(ps_r3, lhsT=one13, rhs=c28, start=True, stop=True)
            rep3 = cp.tile([3, 28], F32, tag="rep3")
            nc.vector.tensor_copy(rep3, ps_r3)
            cdw3 = cp.tile([3, A], F32, tag="cdw3")
            nc.vector.tensor_tensor(cdw3, rep3[:, 4:8], rep3[:, 8:12],
                                    op=OP.add)
            nc.vector.tensor_tensor(cdw3, cdw3, rep3[:, 12:16], op=OP.add)
            nc.vector.tensor_tensor(cdw3, cdw3, rep3[:, 16:20], op=OP.add)
            nc.vector.tensor_tensor(cdw3, cdw3, rep3[:, 20:24], op=OP.add)
            nc.vector.tensor_tensor(cdw3, cdw3, rep3[:, 24:28], op=OP.add)
            bias_all = cp.tile([3, A], F32, tag="bias_all")
            nc.vector.tensor_scalar_mul(bias_all, rep3[:, 0:4], m01_c)
            nc.vector.scalar_tensor_tensor(bias_all, cdw3, m2_c, bias_all,
                                           op0=OP.mult, op1=OP.add)

            # ---- G4 = Wk @ qwyu -> gall (zero-padded) + fp8 copy ------
            gall = cp.tile([128, DC, 48], BF16, tag="gall")
            nc.vector.memset(gall, 0.0)
            for c in range(DC):
                ps_g = sps.tile([128, 16], F32, tag="s")
                nc.tensor.matmul(ps_g, lhsT=wkt_sb[:, c * H:(c + 1) * H],
                                 rhs=qwyu, start=True, stop=True)
                nc.vector.tensor_copy(gall[:, c, 0:16], ps_g)
            g8 = cp.tile([128, DC, 48], F8, tag="g8")
            nc.vector.tensor_copy(g8, gall)
            g8p = g8.rearrange("p (c2 pair) f -> p c2 pair f", pair=2)

            # ---- remaining schedule: JIT-interleaved ------------------
            emit_v_half(0, 1)
            emit_rows(0)
            emit_epi(0)
            for a in range(1, A):
                emit_v_half(a, 0)
                emit_v_half(a, 1)
                emit_rows(a)
                emit_epi(a)

            nc.sync.dma_start(out=out.ap(), in_=attz)

    nc.compile()
    return nc


def _prep_inputs(inputs):
    f = {k: np.asarray(v, dtype=np.float32) for k, v in inputs.items()}
    S = SCALE

    cpackh = np.zeros((128, CH_W), np.float32)
    cpackh[:, CH_WQ:CH_WQ + DC * H] = np.transpose(
        f["Wq"].reshape(DC, 128, H), (1, 0, 2)).reshape(128, DC * H)
    cpackh[:, CH_TAT:CH_TAT + H] = f["trans_W"][:H].T
    cpackh[:, CH_TBT:CH_TBT + H] = f["trans_W"][H:].T
    cpackh[:, CH_W1A:CH_W1A + H] = f["W1_W"][:H]
    cpackh[:, CH_W1BT:CH_W1BT + H] = f["W1_W"][H:].T

    cpackf = np.zeros((128, CF_W), np.float32)
    cpackf[0:4, CF_BQROW:CF_BQROW + H] = np.tile(f["bq"], (4, 1))
    for i, k in enumerate(("bq", "bk", "bv", "W1_b", "trans_b")):
        cpackf[:, CF_BIAS + i] = f[k]
    cpackf[0:3, CF_COMBW] = f["comb_w"]
    cpackf[0:4, CF_MASK4:CF_MASK4 + 4] = np.eye(4)
    cpackf[0:3, CF_M01] = [1.0 / S, 1.0 / S, 0.0]
    cpackf[0:3, CF_M2] = [0.0, 0.0, 1.0 / S]

    cpackb = np.zeros((128, CB_W), np.float32)
    cpackb[:, CB_WKT:CB_WKT + D] = f["Wk"].T
    cpackb[:, CB_WV:CB_WV + DC * H] = np.transpose(
        f["Wv"].reshape(DC, 128, H), (1, 0, 2)).reshape(128, DC * H)
    # combo matrix: ch0(TW): st@32, neg@64; ch1(Wi): sxq@0, neg@64;
    # ch2(DW): sxw@1, sxy@2, sd@99, neg@64
    cpackb[32, CB_COMBO + 0] = 1.0
    cpackb[64, CB_COMBO + 0] = 1.0
    cpackb[0, CB_COMBO + 1] = 1.0
    cpackb[64, CB_COMBO + 1] = 1.0
    cpackb[1, CB_COMBO + 2] = 1.0
    cpackb[2, CB_COMBO + 2] = 1.0
    cpackb[99, CB_COMBO + 2] = 1.0
    cpackb[64, CB_COMBO + 2] = 1.0
    cpackb[0, CB_E0] = 1.0
    cpackb[0:4, CB_ID4:CB_ID4 + 4] = np.eye(4)

    t1 = f["T1"].reshape(H, H * H)
    cpackh_bf = cpackh.astype(BF)
    t1_e4 = np.clip(t1, -240, 240).astype(E4)

    in_maps = []
    for b in range(NCORES):
        ch = cpackh_bf.copy()
        ch[:, CH_ASP:CH_ASP + DC * A] = np.transpose(
            f["aspect_feature"][b].T.reshape(DC, 128, A),
            (1, 0, 2)).reshape(128, DC * A).astype(BF)
        cb = cpackb.copy()
        cb[0:4, CB_MROW:CB_MROW + N] = np.tile(f["fmask"][b], (4, 1))
        m = {"t1f": t1_e4, "cpackh": ch, "cpackf": cpackf,
             "cpackb": cb.astype(BF)}
        xs = np.stack([f["feature"][b], f["all_type_feature"][b]], axis=2)
        # [A, N, 2, D] -> [A, 128(p), DC(c), 2, N]
        m["xt"] = np.ascontiguousarray(
            xs.transpose(0, 3, 2, 1).reshape(A, DC, 128, 2, N)
              .transpose(0, 2, 1, 3, 4)).astype(BF)
        dpt = f["dep_feature"][b].transpose(0, 2, 1).reshape(A, DC, 128, N)
        m["dp8"] = np.clip(np.ascontiguousarray(dpt.transpose(0, 2, 1, 3)),
                           -240, 240).astype(E4).reshape(
                               A, 128, DC // 2, 2, N)
        in_maps.append(m)
    return in_maps


def _install_ntff_shim():
    """Provide antenv.axon_hooks (absent in this image) so trace=True can
    drive NTFF capture through libaxon_pjrt.so."""
    if "antenv.axon_hooks" in sys.modules:
        return
    import antenv

    mod = types.ModuleType("antenv.axon_hooks")
    mod._hook = None
    mod.set_axon_ntff_profile_hook = lambda h: setattr(mod, "_hook", h)
    mod.get_axon_ntff_profile_hook = lambda: mod._hook
    sys.modules["antenv.axon_hooks"] = mod
    antenv.axon_hooks = mod

    so_path = "/opt/axon/libaxon_pjrt.so"
    try:
        lib = ctypes.CDLL(so_path)
    except OSError:
        return
    if not hasattr(lib, "axon_start_nrt_profile"):
        return
    lib.axon_start_nrt_profile.argtypes = [ctypes.POINTER(ctypes.c_int64),
                                           ctypes.c_size_t]
    lib.axon_start_nrt_profile.restype = ctypes.c_int64
    lib.axon_stop_nrt_profile.argtypes = [ctypes.c_char_p]
    lib.axon_stop_nrt_profile.restype = ctypes.c_int64

    @contextlib.contextmanager
    def _hook(output_dir, device_ids):
        import jax

        jax.devices()
        if device_ids:
            ids = (ctypes.c_int64 * len(device_ids))(*device_ids)
            rc = lib.axon_start_nrt_profile(ids, len(device_ids))
        else:
            rc = lib.axon_start_nrt_profile(None, 0)
        if rc != 0:
            raise RuntimeError(f"axon_start_nrt_profile rc={rc}")
        try:
            yield
        finally:
            n = lib.axon_stop_nrt_profile(str(output_dir).encode())
            print(f"profile: {n} file(s) written to {output_dir}")

    mod.set_axon_ntff_profile_hook(_hook)


def kernel(feature, dep_feature, aspect_feature, all_type_feature, fmask,
           Wq, bq, Wk, bk, Wv, bv, trans_W, trans_b, T1, W1_W, W1_b, comb_w,
           _profile=False, _tmpdir=None):
    global LAST_RESULTS
    inputs = dict(feature=feature, dep_feature=dep_feature,
                  aspect_feature=aspect_feature,
                  all_type_feature=all_type_feature, fmask=fmask, Wq=Wq,
                  bq=bq, Wk=Wk, bk=bk, Wv=Wv, bv=bv, trans_W=trans_W,
                  trans_b=trans_b, T1=T1, W1_W=W1_W, W1_b=W1_b,
                  comb_w=comb_w)
    nc = _build()
    in_maps = _prep_inputs(inputs)
    if _profile:
        _install_ntff_shim()
    res = run_bass_kernel_spmd(nc, in_maps, list(range(NCORES)),
                               trace=_profile, tmpdir=_tmpdir)
    LAST_RESULTS = res
    full = np.stack([res.results[c]["out"].T for c in range(NCORES)])
    return full.astype(np.float32)


# revision 14
# speedup vs baseline: 1.0626x; 1.0626x over previous
"""Bass/Tile TRN2 kernel for nn_Disen_GAT_For_Multi_Aspect (v3).

Contract: kernel(**inputs) takes FULL fp32 numpy inputs (keys as in
reference.setup_inputs()) and returns the FULL [B, A, H] fp32 output.

Strategy
--------
Data-parallel over batch B across the 8 cores (1 batch row / core, A=4
aspects per core).  The reference collapses algebraically:

  q = Wq^T asp + bq;  u = tA q; v = tB q; y = W1b v; a3 = W1a^T q
  w[a,k] = sum_{i,j} q[a,i] v[a,j] T1[i,j,k]
  G = Wk @ [q|w|y|u]                    (per aspect, [D,4])
  logit rows vs raw streams:  st = (Wk q).T_n, sx* = (Wk{q,w,y}).X_n,
                              sd = (Wk u).Dp_n
  V_W = Wv^T X + bv, V_T = Wv^T T + bv  ([H,N] per aspect)
  att_z[h] = sum_n att_n V_W[h,n] V_T[h,n]

PE-centric v3 layout:
 * T1 pass computes w DIRECTLY: 128 accumulating matmuls with rank-1
   lhsT chunks qv_j = q (.) v_j (fp8), rhs = T1[:, j-block] (fp8).
 * Per aspect the 5 logit rows land in ONE PSUM bank via tile_position
   col placement (X-rows@0, T-rows@32, Dp-rows@64, neg-row@96), padded
   to M=32 so every PSUM partition is written.  Row matmuls run in
   fp8 DoubleRow mode (K=256 chunk pairs): device-side single-copy
   casts of the bf16 X|T stream and of G to fp8.  One full-width bf16
   copy of the row bank, then softmax as: combo-matmul [128x3] -> ACT
   Exp(bias,scale,accum z) -> reciprocal -> alpha-broadcast matmul =
   att replicated on 128 partitions -> multiply+reduce vs V_W*V_T.
 * Streams: X,T bf16 (V path); rows read fp8 casts; Dp fp8; T1 fp8.
 * q-chain in bf16; constants split hot (q-chain) / cold (stream
   weights) so the first matmul issues as early as possible.
"""

import contextlib
import ctypes
import sys
import types

import numpy as np
import ml_dtypes

import concourse.bacc as bacc
import concourse.mybir as mybir
import concourse.tile as tile
from concourse.bass_utils import run_bass_kernel_spmd

B, A, N, D, H = 8, 4, 512, 1024, 128
SCALE = float(np.sqrt(H))
NCORES = 8
DC = D // H  # 8 contraction chunks of 128

F32 = mybir.dt.float32
BF16 = mybir.dt.bfloat16
F8 = mybir.dt.float8e4
BF = ml_dtypes.bfloat16
E4 = ml_dtypes.float8_e4m3fn
AF = mybir.ActivationFunctionType
OP = mybir.AluOpType
DR = mybir.MatmulPerfMode.DoubleRow

# cpackh (bf16, hot: q-chain weights) column layout
CH_WQ = 0              # [128, 8, 128] Wq chunk-packed
CH_ASP = 1024          # [128, 8, 4] aspect^T chunk-packed
CH_TAT = 1056          # trans_W[:H].T
CH_TBT = 1184          # trans_W[H:].T
CH_W1A = 1312          # W1_W[:H] (raw)
CH_W1BT = 1440         # W1_W[H:].T
CH_W = 1568
# cpackf (f32, small) column layout
CF_BQROW = 0           # rows 0-3: bq as a row [4, 128]
CF_BIAS = 128          # cols: bq|bk|bv|W1_b|trans_b
CF_COMBW = 133         # rows 0-2: comb_w column
CF_MASK4 = 134         # rows 0-3: eye(4) columns
CF_M01 = 138           # [1/S, 1/S, 0] column (rows 0-2)
CF_M2 = 139            # [0, 0, 1/S] column
CF_W = 140
# cpackb (bf16, cold: stream weights) column layout
CB_WKT = 0             # [128, 1024]  Wk^T
CB_WV = 1024           # [128, 8, 128] Wv chunk-packed
CB_MROW = 2048         # rows 0-3: fmask replicated [4, 512]
CB_COMBO = 2560        # [128, 3] combo matrix
CB_E0 = 2563           # [4, 32] one-hot row-0 picker (neg MM lhsT)
CB_ID4 = 2595          # [4, 4] identity
CB_W = 2600

LAST_RESULTS = None  # test harness peeks at this


def _build(ncores=NCORES):
    nc = bacc.Bacc("TRN2", target_bir_lowering=False, debug=False,
                   num_devices=ncores)

    xt = nc.dram_tensor("xt", [A, 128, DC, 2, N], BF16, kind="ExternalInput")
    dp8 = nc.dram_tensor("dp8", [A, 128, DC // 2, 2, N], F8,
                         kind="ExternalInput")
    t1f = nc.dram_tensor("t1f", [H, H * H], F8, kind="ExternalInput")
    cpackh = nc.dram_tensor("cpackh", [128, CH_W], BF16, kind="ExternalInput")
    cpackf = nc.dram_tensor("cpackf", [128, CF_W], F32, kind="ExternalInput")
    cpackb = nc.dram_tensor("cpackb", [128, CB_W], BF16, kind="ExternalInput")
    out = nc.dram_tensor("out", [H, A], F32, kind="ExternalOutput")

    inv_s = 1.0 / SCALE

    with tile.TileContext(nc) as tc:
        with (
            tc.tile_pool(name="const", bufs=1) as cp,
            tc.tile_pool(name="t1s", bufs=2) as tp,
            tc.tile_pool(name="xts", bufs=2) as xp,
            tc.tile_pool(name="work", bufs=2) as wp,
            tc.tile_pool(name="vzone", bufs=4, space="PSUM") as vps,
            tc.tile_pool(name="rzone", bufs=2, space="PSUM") as rps,
            tc.tile_pool(name="szone", bufs=2, space="PSUM") as sps,
        ):
            # ---- input DMAs, spread across the three DMA sequencers ---
            cph = cp.tile([128, CH_W], BF16, tag="cph")
            nc.sync.dma_start(out=cph, in_=cpackh.ap())
            cpf = cp.tile([128, CF_W], F32, tag="cpf")
            nc.scalar.dma_start(out=cpf, in_=cpackf.ap())
            t1sb = []
            for i in range(4):
                t = tp.tile([128, 4096], F8, tag="t1", bufs=4)
                eng = nc.scalar if i % 2 == 0 else nc.sync
                eng.dma_start(out=t, in_=t1f.ap()[:, 4096 * i:4096 * (i + 1)])
                t1sb.append(t)
            cpb = cp.tile([128, CB_W], BF16, tag="cpb")
            nc.scalar.dma_start(out=cpb, in_=cpackb.ap())
            xa_t = {}
            dp_t = {}
            xeng = [nc.gpsimd, nc.sync, nc.scalar, nc.gpsimd]
            deng = [nc.gpsimd, nc.sync, nc.sync, nc.scalar]
            HC = DC // 2
            for a in range(A):
                halves = []
                for hf in range(2):
                    xh = xp.tile([128, HC, 2, N], BF16, tag="xt", bufs=6)
                    xeng[a].dma_start(
                        out=xh, in_=xt.ap()[a, :, hf * HC:(hf + 1) * HC])
                    halves.append(xh)
                da = xp.tile([128, DC // 2, 2, N], F8, tag="dp", bufs=3)
                deng[a].dma_start(out=da, in_=dp8.ap()[a])
                xa_t[a] = halves
                dp_t[a] = da

            # ---- constant views ---------------------------------------
            wq_v = cph[:, CH_WQ:CH_WQ + DC * H].rearrange(
                "p (c h) -> p c h", c=DC)
            asp_v = cph[:, CH_ASP:CH_ASP + DC * A].rearrange(
                "p (c a) -> p c a", c=DC)
            tat_sb = cph[:, CH_TAT:CH_TAT + H]
            tbt_sb = cph[:, CH_TBT:CH_TBT + H]
            w1a_sb = cph[:, CH_W1A:CH_W1A + H]
            w1bt_sb = cph[:, CH_W1BT:CH_W1BT + H]
            bqrow = cpf[0:4, CF_BQROW:CF_BQROW + H]
            bq_c = cpf[:, CF_BIAS + 0:CF_BIAS + 1]
            bk_c = cpf[:, CF_BIAS + 1:CF_BIAS + 2]
            bv_c = cpf[:, CF_BIAS + 2:CF_BIAS + 3]
            b1_c = cpf[:, CF_BIAS + 3:CF_BIAS + 4]
            tb_c = cpf[:, CF_BIAS + 4:CF_BIAS + 5]
            combw3 = cpf[0:3, CF_COMBW:CF_COMBW + 1]
            mask4 = cpf[0:4, CF_MASK4:CF_MASK4 + 4]
            m01_c = cpf[0:3, CF_M01:CF_M01 + 1]
            m2_c = cpf[0:3, CF_M2:CF_M2 + 1]
            wkt_sb = cpb[:, CB_WKT:CB_WKT + D]
            wv_v = cpb[:, CB_WV:CB_WV + DC * H].rearrange(
                "p (c h) -> p c h", c=DC)
            mrow4 = cpb[0:4, CB_MROW:CB_MROW + N]
            combo_m = cpb[:, CB_COMBO:CB_COMBO + 3]
            id4 = cpb[0:4, CB_ID4:CB_ID4 + 4]

            ones_col = cp.tile([128, 1], F32, tag="ones_col")
            nc.vector.memset(ones_col, 1.0)
            ones3r = cp.tile([3, 128], BF16, tag="ones3r")
            nc.vector.memset(ones3r, 1.0)

            # ---- q chain (bf16 matmuls, fp32 psum) --------------------
            ps_q = sps.tile([H, A], F32, tag="s")
            for c in range(DC):
                nc.tensor.matmul(ps_q, lhsT=wq_v[:, c, :], rhs=asp_v[:, c, :],
                                 start=(c == 0), stop=(c == DC - 1))
            q4 = cp.tile([H, A], F32, tag="q4")
            nc.scalar.activation(q4, ps_q, AF.Identity, bias=bq_c)
            q4b = cp.tile([H, A], BF16, tag="q4b")
            nc.vector.tensor_copy(q4b, q4)

            ps_qT = sps.tile([A, H], F32, tag="s")
            for c in range(DC):
                nc.tensor.matmul(ps_qT, lhsT=asp_v[:, c, :], rhs=wq_v[:, c, :],
                                 start=(c == 0), stop=(c == DC - 1))
            qTb = cp.tile([A, H], BF16, tag="qTb")
            nc.vector.tensor_tensor(qTb, ps_qT, bqrow, op=OP.add)

            ps_s = sps.tile([H, A], F32, tag="s")
            nc.tensor.matmul(ps_s, lhsT=tbt_sb, rhs=q4b, start=True, stop=True)
            v4 = cp.tile([H, A], F32, tag="v4")
            nc.vector.tensor_copy(v4, ps_s)
            v4b = cp.tile([H, A], BF16, tag="v4b")
            nc.vector.tensor_copy(v4b, ps_s)

            ps_vT = sps.tile([A, H], F32, tag="s")
            nc.tensor.matmul(ps_vT, lhsT=q4b, rhs=tbt_sb, start=True,
                             stop=True)
            vTb = cp.tile([A, H], BF16, tag="vTb")
            nc.vector.tensor_copy(vTb, ps_vT)

            # qwyu: aspect-major columns [q|w|y|u] per aspect, bf16
            qwyu = cp.tile([H, 16], BF16, tag="qwyu")
            qwv = qwyu.rearrange("p (a v) -> p a v", a=4)
            nc.vector.tensor_copy(qwv[:, :, 0], q4)

            ps_s = sps.tile([H, A], F32, tag="s")
            nc.tensor.matmul(ps_s, lhsT=tat_sb, rhs=q4b, start=True, stop=True)
            u4 = cp.tile([H, A], F32, tag="u4")
            nc.vector.tensor_copy(u4, ps_s)
            nc.vector.tensor_copy(qwv[:, :, 3], ps_s)

            ps_s = sps.tile([H, A], F32, tag="s")
            nc.tensor.matmul(ps_s, lhsT=w1bt_sb, rhs=v4b, start=True,
                             stop=True)
            y4 = cp.tile([H, A], F32, tag="y4")
            nc.vector.tensor_copy(y4, ps_s)
            nc.vector.tensor_copy(qwv[:, :, 2], ps_s)

            ps_s = sps.tile([H, A], F32, tag="s")
            nc.tensor.matmul(ps_s, lhsT=w1a_sb, rhs=q4b, start=True, stop=True)
            a3q = cp.tile([H, A], F32, tag="a3q")
            nc.vector.tensor_copy(a3q, ps_s)

            # ---- qv outer products (masked K=4), cast fp8 -------------
            ps_qv = sps.tile([128, 4 * H], F32, tag="s")
            for a in range(A):
                vTm = wp.tile([A, H], BF16, tag="vTm")
                nc.vector.tensor_scalar_mul(vTm, vTb, mask4[:, a:a + 1])
                nc.tensor.matmul(ps_qv[:, a * H:(a + 1) * H], lhsT=qTb,
                                 rhs=vTm, start=True, stop=True)
            qv8 = cp.tile([128, 4 * H], F8, tag="qv8")
            nc.vector.tensor_copy(qv8, ps_qv)
            qv8v = qv8.rearrange("p (a j) -> p j a", a=4)

            # ---- helpers for interleaved emission ---------------------
            vps_t = {}
            vv_t = {}
            pp_t = {}
            rm_t = {}

            def emit_t1_block(blk, ps_w):
                for j in range(32 * blk, 32 * blk + 32):
                    nc.tensor.matmul(ps_w, lhsT=qv8v[:, j, :],
                                     rhs=t1sb[j // 32][:, (j % 32) * H:
                                                       (j % 32 + 1) * H],
                                     start=(j == 0), stop=(j == H - 1))

            def emit_v_half(a, hf):
                if hf == 0:
                    ps_vw = vps.tile([H, N], F32, tag="v")
                    ps_vt = vps.tile([H, N], F32, tag="v")
                    vps_t[a] = (ps_vw, ps_vt)
                ps_vw, ps_vt = vps_t[a]
                xh = xa_t[a][hf]
                for c in range(HC):
                    cc = hf * HC + c
                    nc.tensor.matmul(ps_vw, lhsT=wv_v[:, cc, :],
                                     rhs=xh[:, c, 0, :], start=(cc == 0),
                                     stop=(cc == DC - 1))
                    nc.tensor.matmul(ps_vt, lhsT=wv_v[:, cc, :],
                                     rhs=xh[:, c, 1, :], start=(cc == 0),
                                     stop=(cc == DC - 1))
                if hf == 1:
                    vv = wp.tile([H, 2 * N], F32, tag="vv")
                    nc.scalar.activation(vv[:, 0:N], ps_vw, AF.Identity,
                                         bias=bv_c)
                    nc.scalar.activation(vv[:, N:2 * N], ps_vt, AF.Identity,
                                         bias=bv_c)
                    pprod = wp.tile([H, N], F32, tag="pprod")
                    nc.vector.tensor_mul(pprod, vv[:, 0:N], vv[:, N:2 * N])
                    vv_t[a] = vv
                    pp_t[a] = pprod

            def emit_rows(a):
                da = dp_t[a]
                ps_rm = rps.tile([128, N], F32, tag="rows", bufs=1)
                for st in range(2):
                    for c in range(DC):
                        xh = xa_t[a][c // HC]
                        nc.tensor.matmul(ps_rm[32 * st:32 * st + 32, :],
                                         lhsT=gall[:, c, 4 * a:4 * a + 32],
                                         rhs=xh[:, c % HC, st, :],
                                         start=(c == 0), stop=(c == DC - 1),
                                         tile_position=(0, 32 * st))
                ps_rd = rps.tile([32, N], F32, tag="rowsd", bufs=1)
                for c2 in range(DC // 2):
                    nc.tensor.matmul(ps_rd,
                                     lhsT=g8p[:, c2, :, 4 * a:4 * a + 32],
                                     rhs=da[:, c2, :, :], start=(c2 == 0),
                                     stop=(c2 == DC // 2 - 1),
                                     perf_mode=DR)
                rm_t[a] = (ps_rm, ps_rd)

            def emit_epi(a):
                ps_rm, ps_rd = rm_t[a]
                # assemble rows bank: X@0, T@32, neg@64, Dp@96
                rows_bf = wp.tile([128, N], BF16, tag="rows_bf")
                nc.vector.tensor_copy(rows_bf[0:64, :], ps_rm[0:64, :])
                nc.scalar.copy(rows_bf[64:96, :], negfull)
                nc.vector.tensor_copy(rows_bf[96:128, :], ps_rd)
                ps_combo = sps.tile([3, N], F32, tag="s")
                nc.tensor.matmul(ps_combo, lhsT=combo_m, rhs=rows_bf,
                                 start=True, stop=True)
                e3 = wp.tile([3, N], BF16, tag="e3")
                z3 = wp.tile([3, 1], F32, tag="z3")
                nc.scalar.activation(e3, ps_combo, AF.Exp,
                                     bias=bias_all[:, a:a + 1], scale=inv_s,
                                     accum_out=z3)
                rz = wp.tile([3, 1], F32, tag="rz")
                nc.vector.reciprocal(rz, z3)
                alpha = wp.tile([3, 1], F32, tag="alpha")
                nc.vector.tensor_mul(alpha, rz, combw3)
                arep = wp.tile([3, H], BF16, tag="arep")
                nc.vector.tensor_scalar_mul(arep, ones3r, alpha)
                ps_att = sps.tile([H, N], F32, tag="s")
                nc.tensor.matmul(ps_att, lhsT=arep, rhs=e3,
                                 start=True, stop=True)
                scr = wp.tile([H, N], F32, tag="scr")
                nc.vector.tensor_mul(scr, ps_att, pp_t[a], )
                nc.vector.tensor_reduce(attz[:, a:a + 1], scr,
                                        axis=mybir.AxisListType.X,
                                        op=OP.add)

            attz = cp.tile([H, A], F32, tag="attz")

            # ---- T1 pass interleaved with aspect-0 V work -------------
            ps_w = sps.tile([A, H], F32, tag="s")
            emit_t1_block(0, ps_w)
            emit_t1_block(1, ps_w)
            emit_t1_block(2, ps_w)
            emit_v_half(0, 0)
            emit_t1_block(3, ps_w)

            wbf = cp.tile([A, H], BF16, tag="wbf")
            nc.vector.tensor_copy(wbf, ps_w)
            ps_tr = sps.tile([H, A], BF16, tag="s")
            nc.tensor.transpose(ps_tr, wbf, id4)
            nc.vector.tensor_copy(qwv[:, :, 1], ps_tr)

            # ---- scalar terms -> bias_all [3, A] ----------------------
            # groups: cbk | u.bk | w.bk | y.bk | a3.v | v.W1b | q.tb
            tmp28 = cp.tile([H, 28], F32, tag="tmp28")
            nc.vector.tensor_scalar_mul(tmp28[:, 0:4], q4, bk_c)
            nc.vector.tensor_scalar_mul(tmp28[:, 4:8], u4, bk_c)
            wcol = cp.tile([H, A], F32, tag="wcol")
            nc.vector.tensor_copy(wcol, ps_tr)
            nc.vector.tensor_scalar_mul(tmp28[:, 8:12], wcol, bk_c)
            nc.vector.tensor_scalar_mul(tmp28[:, 12:16], y4, bk_c)
            nc.vector.tensor_mul(tmp28[:, 16:20], a3q, v4)
            nc.vector.tensor_scalar_mul(tmp28[:, 20:24], v4, b1_c)
            nc.vector.tensor_scalar_mul(tmp28[:, 24:28], q4, tb_c)
            ps_c28 = sps.tile([1, 28], F32, tag="s")
            nc.tensor.matmul(ps_c28, lhsT=ones_col, rhs=tmp28,
                             start=True, stop=True)
            c28 = cp.tile([1, 28], F32, tag="c28")
            nc.vector.tensor_copy(c28, ps_c28)
            one13 = cp.tile([1, 3], F32, tag="one13")
            nc.vector.memset(one13, 1.0)
            ps_r3 = sps.tile([3, 28], F32, tag="s")
            nc.tensor.matmul(ps_r3, lhsT=one13, rhs=c28, start=True, stop=True)
            rep3 = cp.tile([3, 28], F32, tag="rep3")
            nc.vector.tensor_copy(rep3, ps_r3)
            cdw3 = cp.tile([3, A], F32, tag="cdw3")
            nc.vector.tensor_tensor(cdw3, rep3[:, 4:8], rep3[:, 8:12],
                                    op=OP.add)
            nc.vector.tensor_tensor(cdw3, cdw3, rep3[:, 12:16], op=OP.add)
            nc.vector.tensor_tensor(cdw3, cdw3, rep3[:, 16:20], op=OP.add)
            nc.vector.tensor_tensor(cdw3, cdw3, rep3[:, 20:24], op=OP.add)
            nc.vector.tensor_tensor(cdw3, cdw3, rep3[:, 24:28], op=OP.add)
            bias_all = cp.tile([3, A], F32, tag="bias_all")
            nc.vector.tensor_scalar_mul(bias_all, rep3[:, 0:4], m01_c)
            nc.vector.scalar_tensor_tensor(bias_all, cdw3, m2_c, bias_all,
                                           op0=OP.mult, op1=OP.add)

            # ---- G4 = Wk @ qwyu -> gall (zero-padded) + fp8 copy ------
            gall = cp.tile([128, DC, 48], BF16, tag="gall")
            nc.vector.memset(gall, 0.0)
            for c in range(DC):
                ps_g = sps.tile([128, 16], F32, tag="s")
                nc.tensor.matmul(ps_g, lhsT=wkt_sb[:, c * H:(c + 1) * H],
                                 rhs=qwyu, start=True, stop=True)
                nc.vector.tensor_copy(gall[:, c, 0:16], ps_g)
            g8 = cp.tile([128, DC, 48], F8, tag="g8")
            nc.vector.tensor_copy(g8, gall)
            g8p = g8.rearrange("p (c2 pair) f -> p c2 pair f", pair=2)

            # neg rows: [32, N] with row 0 = -1e30*(1-m), rest zero
            negfull = cp.tile([32, N], BF16, tag="negfull")
            nc.vector.memset(negfull, 0.0)
            nc.vector.tensor_scalar(negfull[0:1, :], mrow4[0:1, :], 1e30,
                                    1e30, op0=OP.mult, op1=OP.subtract)

            # ---- remaining schedule: JIT-interleaved ------------------
            emit_v_half(0, 1)
            emit_rows(0)
            emit_epi(0)
            for a in range(1, A):
                emit_v_half(a, 0)
                emit_v_half(a, 1)
                emit_rows(a)
                emit_epi(a)

            nc.sync.dma_start(out=out.ap(), in_=attz)

    nc.compile()
    return nc


def _prep_inputs(inputs):
    f = {k: np.asarray(v, dtype=np.float32) for k, v in inputs.items()}
    S = SCALE

    cpackh = np.zeros((128, CH_W), np.float32)
    cpackh[:, CH_WQ:CH_WQ + DC * H] = np.transpose(
        f["Wq"].reshape(DC, 128, H), (1, 0, 2)).reshape(128, DC * H)
    cpackh[:, CH_TAT:CH_TAT + H] = f["trans_W"][:H].T
    cpackh[:, CH_TBT:CH_TBT + H] = f["trans_W"][H:].T
    cpackh[:, CH_W1A:CH_W1A + H] = f["W1_W"][:H]
    cpackh[:, CH_W1BT:CH_W1BT + H] = f["W1_W"][H:].T

    cpackf = np.zeros((128, CF_W), np.float32)
    cpackf[0:4, CF_BQROW:CF_BQROW + H] = np.tile(f["bq"], (4, 1))
    for i, k in enumerate(("bq", "bk", "bv", "W1_b", "trans_b")):
        cpackf[:, CF_BIAS + i] = f[k]
    cpackf[0:3, CF_COMBW] = f["comb_w"]
    cpackf[0:4, CF_MASK4:CF_MASK4 + 4] = np.eye(4)
    cpackf[0:3, CF_M01] = [1.0 / S, 1.0 / S, 0.0]
    cpackf[0:3, CF_M2] = [0.0, 0.0, 1.0 / S]

    cpackb = np.zeros((128, CB_W), np.float32)
    cpackb[:, CB_WKT:CB_WKT + D] = f["Wk"].T
    cpackb[:, CB_WV:CB_WV + DC * H] = np.transpose(
        f["Wv"].reshape(DC, 128, H), (1, 0, 2)).reshape(128, DC * H)
    # combo matrix: ch0(TW): st@32, neg@64; ch1(Wi): sxq@0, neg@64;
    # ch2(DW): sxw@1, sxy@2, sd@99, neg@64
    cpackb[32, CB_COMBO + 0] = 1.0
    cpackb[64, CB_COMBO + 0] = 1.0
    cpackb[0, CB_COMBO + 1] = 1.0
    cpackb[64, CB_COMBO + 1] = 1.0
    cpackb[1, CB_COMBO + 2] = 1.0
    cpackb[2, CB_COMBO + 2] = 1.0
    cpackb[99, CB_COMBO + 2] = 1.0
    cpackb[64, CB_COMBO + 2] = 1.0
    cpackb[0, CB_E0] = 1.0
    cpackb[0:4, CB_ID4:CB_ID4 + 4] = np.eye(4)

    t1 = f["T1"].reshape(H, H * H)
    cpackh_bf = cpackh.astype(BF)
    t1_e4 = np.clip(t1, -240, 240).astype(E4)

    in_maps = []
    for b in range(NCORES):
        ch = cpackh_bf.copy()
        ch[:, CH_ASP:CH_ASP + DC * A] = np.transpose(
            f["aspect_feature"][b].T.reshape(DC, 128, A),
            (1, 0, 2)).reshape(128, DC * A).astype(BF)
        cb = cpackb.copy()
        cb[0:4, CB_MROW:CB_MROW + N] = np.tile(f["fmask"][b], (4, 1))
        m = {"t1f": t1_e4, "cpackh": ch, "cpackf": cpackf,
             "cpackb": cb.astype(BF)}
        xs = np.stack([f["feature"][b], f["all_type_feature"][b]], axis=2)
        # [A, N, 2, D] -> [A, 128(p), DC(c), 2, N]
        m["xt"] = np.ascontiguousarray(
            xs.transpose(0, 3, 2, 1).reshape(A, DC, 128, 2, N)
              .transpose(0, 2, 1, 3, 4)).astype(BF)
        dpt = f["dep_feature"][b].transpose(0, 2, 1).reshape(A, DC, 128, N)
        m["dp8"] = np.clip(np.ascontiguousarray(dpt.transpose(0, 2, 1, 3)),
                           -240, 240).astype(E4).reshape(
                               A, 128, DC // 2, 2, N)
        in_maps.append(m)
    return in_maps


def _install_ntff_shim():
    """Provide antenv.axon_hooks (absent in this image) so trace=True can
    drive NTFF capture through libaxon_pjrt.so."""
    if "antenv.axon_hooks" in sys.modules:
        return
    import antenv

    mod = types.ModuleType("antenv.axon_hooks")
    mod._hook = None
    mod.set_axon_ntff_profile_hook = lambda h: setattr(mod, "_hook", h)
    mod.get_axon_ntff_profile_hook = lambda: mod._hook
    sys.modules["antenv.axon_hooks"] = mod
    antenv.axon_hooks = mod

    so_path = "/opt/axon/libaxon_pjrt.so"
    try:
        lib = ctypes.CDLL(so_path)
    except OSError:
        return
    if not hasattr(lib, "axon_start_nrt_profile"):
        return
    lib.axon_start_nrt_profile.argtypes = [ctypes.POINTER(ctypes.c_int64),
                                           ctypes.c_size_t]
    lib.axon_start_nrt_profile.restype = ctypes.c_int64
    lib.axon_stop_nrt_profile.argtypes = [ctypes.c_char_p]
    lib.axon_stop_nrt_profile.restype = ctypes.c_int64

    @contextlib.contextmanager
    def _hook(output_dir, device_ids):
        import jax

        jax.devices()
        if device_ids:
            ids = (ctypes.c_int64 * len(device_ids))(*device_ids)
            rc = lib.axon_start_nrt_profile(ids, len(device_ids))
        else:
            rc = lib.axon_start_nrt_profile(None, 0)
        if rc != 0:
            raise RuntimeError(f"axon_start_nrt_profile rc={rc}")
        try:
            yield
        finally:
            n = lib.axon_stop_nrt_profile(str(output_dir).encode())
            print(f"profile: {n} file(s) written to {output_dir}")

    mod.set_axon_ntff_profile_hook(_hook)


def kernel(feature, dep_feature, aspect_feature, all_type_feature, fmask,
           Wq, bq, Wk, bk, Wv, bv, trans_W, trans_b, T1, W1_W, W1_b, comb_w,
           _profile=False, _tmpdir=None):
    global LAST_RESULTS
    inputs = dict(feature=feature, dep_feature=dep_feature,
                  aspect_feature=aspect_feature,
                  all_type_feature=all_type_feature, fmask=fmask, Wq=Wq,
                  bq=bq, Wk=Wk, bk=bk, Wv=Wv, bv=bv, trans_W=trans_W,
                  trans_b=trans_b, T1=T1, W1_W=W1_W, W1_b=W1_b,
                  comb_w=comb_w)
    nc = _build()
    in_maps = _prep_inputs(inputs)
    if _profile:
        _install_ntff_shim()
    res = run_bass_kernel_spmd(nc, in_maps, list(range(NCORES)),
                               trace=_profile, tmpdir=_tmpdir)
    LAST_RESULTS = res
    full = np.stack([res.results[c]["out"].T for c in range(NCORES)])
    return full.astype(np.float32)
